# revision 1
# baseline (speedup 1.0000x reference)
"""Trainium2 Bass kernel for nn_CDEM_62079457296798 (channel-attention
transformer block).

Sharding: 8 cores = 4 batches x 2 spatial halves (64 rows + 1 halo row each).
Cross-core communication: one small AllReduce per core-pair carrying the
channel-attention Gram matrices and q/k l2-norm sums; everything else local.

Layout: channel-major activations [C_part, pixels_free]. The attention path
(q/kv convs, depthwise 3x3, Gram, z) uses per-head channel padding 48 -> 64
(256 padded channels) so head boundaries are 32/64 aligned, and runs in bf16.
The depthwise 3x3 runs on the tensor engine as 9 accumulated diag-block
matmuls per 32-channel group with 16-way tile_position packing. The trunk
(linear/ffn/proj) runs in float32r (full-rate fp32).
"""
import sys
sys.path.insert(0, '/opt/trn_rl_repo')

import numpy as np
import ml_dtypes

from concourse import bacc, mybir, tile
from concourse.bass import _add_dep_helper
from concourse.bass_utils import run_bass_kernel_spmd

F32 = mybir.dt.float32
F32R = mybir.dt.float32r
BF16 = mybir.dt.bfloat16
AF = mybir.ActivationFunctionType
OP = mybir.AluOpType
bf16 = ml_dtypes.bfloat16

N_CORES = 8
B, C, H, W = 4, 192, 128, 128
HEADS, CH = 4, 48
CPH = 64                # padded channels per head
CP = HEADS * CPH        # 256 padded attn channels
HLOC = 64               # image rows per core
ER, EC = 66, 130        # ext rows/cols (halo + zero pad)
NEXT = ER * EC          # 8580
NLOC = HLOC * W         # 8192
NCK = 16                # output chunks (4 rows x 128 = 512 px)
CONV_CHUNKS = [(i * 512, 512) for i in range(16)] + [(16 * 512, NEXT - 16 * 512)]
GRP = 2048
CONV_GROUPS = [(i * GRP, GRP) for i in range(4)] + [(4 * GRP, NEXT - 4 * GRP)]
KB = [(0, 128), (128, 64)]          # 192-channel K bands

DIRECT_PSUM_OUT = False  # DMA final result straight from PSUM


import os
STAGE = int(os.environ.get("KSTAGE", "4"))
KSUB = int(os.environ.get("KSUB", "4"))


class _StageDone(Exception):
    pass


def build_nc():
    nc = bacc.Bacc("TRN2", target_bir_lowering=False, debug=False,
                   num_devices=N_CORES)

    d_xe = nc.dram_tensor("xe", [C, NEXT], BF16, kind="ExternalInput")
    d_ye = nc.dram_tensor("ye", [C, NEXT], BF16, kind="ExternalInput")
    d_yc = nc.dram_tensor("yc", [C, NLOC], F32, kind="ExternalInput")
    d_wq = nc.dram_tensor("wq", [C, CP], BF16, kind="ExternalInput")
    d_wkv = nc.dram_tensor("wkv", [C, 2 * CP], BF16, kind="ExternalInput")
    d_qdw = nc.dram_tensor("qdw", [CP, 9, 32], BF16, kind="ExternalInput")
    d_kvdw = nc.dram_tensor("kvdw", [2 * CP, 9, 32], BF16, kind="ExternalInput")
    d_wlin = nc.dram_tensor("wlin", [CP, C], BF16, kind="ExternalInput")
    d_wf1 = nc.dram_tensor("wf1", [C, 768], BF16, kind="ExternalInput")
    d_wf2 = nc.dram_tensor("wf2", [768, C], BF16, kind="ExternalInput")
    d_wpr = nc.dram_tensor("wpr", [C, C], BF16, kind="ExternalInput")
    d_tempb = nc.dram_tensor("tempb", [128, 2], F32, kind="ExternalInput")
    d_alpha = nc.dram_tensor("alpha", [128, 1], F32, kind="ExternalInput")
    d_gamma = nc.dram_tensor("gamma", [128, 1], F32, kind="ExternalInput")
    d_id128 = nc.dram_tensor("id128", [128, 128], F32, kind="ExternalInput")
    d_idrep = nc.dram_tensor("idrep", [128, 64], F32, kind="ExternalInput")
    d_out = nc.dram_tensor("out", [C, NLOC], F32, kind="ExternalOutput")
    d_attn = nc.dram_tensor("attn_bounce", [2, 2, 48, 48], BF16)
    cc_in = nc.dram_tensor("cc_in", [112, 228], F32)
    cc_out = nc.dram_tensor("cc_out", [112, 228], F32)

    with tile.TileContext(nc) as tc:
        with (
            tc.tile_pool(name="sbw", bufs=1) as sbw,      # weights/consts
            tc.tile_pool(name="sbpre", bufs=1) as sbpre,  # conv1x1 out (ext img)
            tc.tile_pool(name="sbin", bufs=3) as sbin,    # streamed conv inputs
            tc.tile_pool(name="sbqk", bufs=4) as sbqk,    # q/k chunk tiles
            tc.tile_pool(name="sbT", bufs=1) as sbT,      # qT/kT/v persistents
            tc.tile_pool(name="sbs", bufs=1) as sbs,      # small attn tiles
            tc.tile_pool(name="sbc", bufs=2) as sbc,      # trunk chunk pipeline
            tc.tile_pool(name="sbg", bufs=6) as sbg,      # gelu chunk tiles
            tc.tile_pool(name="pcv", bufs=4, space="PSUM") as pcv,
            tc.tile_pool(name="pdw", bufs=2, space="PSUM") as pdw,
            tc.tile_pool(name="pacc", bufs=1, space="PSUM") as pacc,
            tc.tile_pool(name="psm", bufs=1, space="PSUM") as psm,
        ):
            # ---------- weights ----------
            wq_t = [sbw.tile([s, CP], BF16, tag=f"wq{i}", name=f"wq{i}")
                    for i, (o, s) in enumerate(KB)]
            wkv_t = [sbw.tile([s, 2 * CP], BF16, tag=f"wkv{i}", name=f"wkv{i}")
                     for i, (o, s) in enumerate(KB)]
            for i, (o, s) in enumerate(KB):
                nc.sync.dma_start(wq_t[i][:], d_wq[o:o + s, :])
            qdw_t = [sbw.tile([128, 9, 32], BF16, tag=f"qdw{m}", name=f"qdw{m}") for m in range(2)]
            kvdw_t = [sbw.tile([128, 9, 32], BF16, tag=f"kvdw{m}", name=f"kvdw{m}") for m in range(4)]
            for m in range(2):
                nc.sync.dma_start(qdw_t[m][:], d_qdw[128 * m:128 * (m + 1)])

            def load_kv_weights():
                for i, (o, s) in enumerate(KB):
                    nc.sync.dma_start(wkv_t[i][:], d_wkv[o:o + s, :])
                for m in range(4):
                    nc.sync.dma_start(kvdw_t[m][:], d_kvdw[128 * m:128 * (m + 1)])
            wlin_t = [sbw.tile([128, C], BF16, tag=f"wlin{m}", name=f"wlin{m}") for m in range(2)]
            wf1_t = [sbw.tile([s, 768], BF16, tag=f"wf1{i}", name=f"wf1{i}")
                     for i, (o, s) in enumerate(KB)]
            wf2_t = [sbw.tile([128, C], BF16, tag=f"wf2{k}", name=f"wf2{k}") for k in range(6)]
            wpr_t = [sbw.tile([s, C], BF16, tag=f"wpr{i}", name=f"wpr{i}")
                     for i, (o, s) in enumerate(KB)]
            tempb = sbw.tile([128, 2], F32, tag="tempb", name="tempb")
            alphav = sbw.tile([128, 1], F32, tag="alphav", name="alphav")
            gammav = sbw.tile([128, 1], F32, tag="gammav", name="gammav")
            id128 = sbw.tile([128, 128], F32, tag="id128", name="id128")
            idrep = sbw.tile([128, 64], F32, tag="idrep", name="idrep")

            def load_trunk_weights():
                for m in range(2):
                    nc.sync.dma_start(wlin_t[m][:], d_wlin[128 * m:128 * (m + 1), :])
                for i, (o, s) in enumerate(KB):
                    nc.sync.dma_start(wf1_t[i][:], d_wf1[o:o + s, :])
                for k in range(6):
                    nc.sync.dma_start(wf2_t[k][:], d_wf2[128 * k:128 * (k + 1), :])
                for i, (o, s) in enumerate(KB):
                    nc.sync.dma_start(wpr_t[i][:], d_wpr[o:o + s, :])
                nc.sync.dma_start(tempb[:], d_tempb.ap())
                nc.sync.dma_start(alphav[:], d_alpha.ap())
                nc.sync.dma_start(gammav[:], d_gamma.ap())
                nc.sync.dma_start(id128[:], d_id128.ap())
                nc.sync.dma_start(idrep[:], d_idrep.ap())

            # persistent attn-path results
            qT = [sbT.tile([128, 64, 112], BF16, tag=f"qT{p}", name=f"qT{p}") for p in range(2)]
            kT = [sbT.tile([128, 64, 112], BF16, tag=f"kT{p}", name=f"kT{p}") for p in range(2)]
            vband = [sbT.tile([128, NLOC], BF16, tag=f"v{m}", name=f"v{m}") for m in range(2)]
            sqp = [sbs.tile([128, NCK], F32, tag=f"sqp{i}", name=f"sqp{i}") for i in range(4)]
            for i in range(4):
                nc.vector.memset(sqp[i][:], 0.0)
            gacc = pacc.tile([112, 224], F32, tag="gacc", name="gacc")

            # ============ q/k/v production ============
            def conv_dw_path(src_dram, w_t, dw_tiles, n_mb, sink, m_off=0,
                             collect_mms=None):
                """192 -> n_mb*128 padded conv1x1 + depthwise 3x3 per band.
                sink(m, ck, psum_flat_ap) consumes each dwconv chunk."""
                for m in range(m_off, m_off + n_mb):
                    pre = sbpre.tile([128, ER, EC], BF16, tag="pre", name="pre")
                    pref = pre[:].rearrange("p a b -> p (a b)")
                    ci = 0
                    for g0, gn in CONV_GROUPS:
                        xc = [sbin.tile([s, GRP], BF16, tag=f"xin{i}", name=f"xin{i}")
                              for i, (o, s) in enumerate(KB)]
                        for i, (o, s) in enumerate(KB):
                            nc.sync.dma_start(xc[i][:, :gn],
                                              src_dram[o:o + s, g0:g0 + gn])
                        for c0 in range(0, gn, 512):
                            cn = min(512, gn - c0)
                            ps = pcv.tile([128, 512], F32, tag="cv", name="cv")
                            for i in range(2):
                                mm = nc.tensor.matmul(
                                    ps[:, :cn],
                                    w_t[i][:, 128 * m:128 * (m + 1)],
                                    xc[i][:, c0:c0 + cn],
                                    start=(i == 0), stop=(i == 1))
                                if collect_mms is not None:
                                    collect_mms.append(mm)
                            if ci % 2 == 0:
                                nc.vector.tensor_copy(pref[:, g0 + c0:g0 + c0 + cn],
                                                      ps[:, :cn])
                            else:
                                nc.scalar.copy(pref[:, g0 + c0:g0 + c0 + cn],
                                               ps[:, :cn])
                            ci += 1
                    for ck in range(NCK if KSUB >= 2 else 0):
                        r0 = 1 + 4 * ck
                        dp = pdw.tile([128, 4, 128], F32, tag="dw", name="dw")
                        for t in range(9):
                            dr, dc = t // 3 - 1, t % 3 - 1
                            for g in range(4):
                                nc.tensor.matmul(
                                    dp[g * 32:(g + 1) * 32, :, :],
                                    dw_tiles[m][g * 32:(g + 1) * 32, t, :],
                                    pre[g * 32:(g + 1) * 32,
                                        r0 + dr:r0 + 4 + dr, 1 + dc:129 + dc],
                                    start=(t == 0), stop=(t == 8),
                                    tile_position=(g * 32, g * 32))
                        sink(m, ck, dp[:].rearrange("p a b -> p (a b)"))

            def qk_sink(dstT, sq_idx):
                qcbig = [None]

                def sink(m, ck, flat):
                    j = ck % 4
                    if j == 0:
                        qcbig[0] = sbqk.tile([128, 2048], BF16, tag="qkc", name="qkc")
                    qc = qcbig[0][:, 512 * j:512 * (j + 1)]
                    if ck % 2 == 0:
                        nc.vector.tensor_copy(qc, flat)
                    else:
                        nc.scalar.copy(qc, flat)
                    if KSUB >= 3:
                        dmp = sbqk.tile([128, 512], F32, tag="dump", name="dump")
                        nc.scalar.activation(dmp[:], qc, AF.Square,
                                             accum_out=sqp[sq_idx + m][:, ck:ck + 1])
                    if KSUB >= 4 and j == 3:
                        nc.sync.dma_start_transpose(
                            dstT[m][:, 4 * ck - 12:4 * ck + 4, :],
                            qcbig[0][0:112, :])
                return sink

            load_kv_weights()
            conv_dw_path(d_xe, wq_t, qdw_t, 2, qk_sink(qT, 0))

            if STAGE >= 2:

                def v_sink(m, ck, flat):
                    dst = vband[m - 2]
                    if ck % 2 == 0:
                        nc.vector.tensor_copy(dst[:, ck * 512:(ck + 1) * 512], flat)
                    else:
                        nc.scalar.copy(dst[:, ck * 512:(ck + 1) * 512], flat)

                conv_dw_path(d_ye, wkv_t, kvdw_t, 2, qk_sink(kT, 2))
                load_trunk_weights()

            # ============ Gram accumulation ============
            gram_last = []
            for p in range(STAGE >= 2 and 2 or 0):
                for ck in range(64):
                    gmm = nc.tensor.matmul(gacc[:, 112 * p:112 * (p + 1)],
                                           qT[p][:, ck, :], kT[p][:, ck, :],
                                           start=(ck == 0), stop=(ck == 63))
                    if ck == 63:
                        gram_last.append(gmm)

            if STAGE < 3:
                oc0 = sbs.tile([128, NCK], F32, tag="oc0d", name="oc0d")
                nc.vector.tensor_copy(oc0[:], sqp[0][:])
                nc.sync.dma_start(d_out[0:128, 0:NCK], oc0[:])
            if STAGE >= 3:
                sqv = sbs.tile([128, 2], F32, tag="sqv", name="sqv")
                skv = sbs.tile([128, 2], F32, tag="skv", name="skv")
                for m in range(2):
                    nc.vector.tensor_reduce(sqv[:, m:m + 1], sqp[m][:],
                                            axis=mybir.AxisListType.X, op=OP.add)
                    nc.vector.tensor_reduce(skv[:, m:m + 1], sqp[2 + m][:],
                                            axis=mybir.AxisListType.X, op=OP.add)

                # ============ pair AllReduce ============
                gsb = sbs.tile([112, 224], F32, tag="gsb", name="gsb")
                nc.vector.tensor_copy(gsb[:], gacc[:])
                nc.scalar.dma_start(cc_in.ap()[:, 0:224], gsb[:])
                nc.scalar.dma_start(cc_in.ap()[:, 224:226], sqv[0:112, :])
                nc.scalar.dma_start(cc_in.ap()[:, 226:228], skv[0:112, :])
                nc.gpsimd.collective_compute(
                    "AllReduce", OP.add,
                    replica_groups=[[0, 1], [2, 3], [4, 5], [6, 7]],
                    ins=[cc_in.ap()], outs=[cc_out.ap()])
                # v(m2) here so it overlaps the AllReduce + softmax
                v_mms = []
                conv_dw_path(d_ye, wkv_t, kvdw_t, 1, v_sink, m_off=2,
                             collect_mms=v_mms)
                gg = sbs.tile([112, 224], F32, tag="gg", name="gg")
                sqg = sbs.tile([128, 2], F32, tag="sqg", name="sqg")
                skg = sbs.tile([128, 2], F32, tag="skg", name="skg")
                nc.vector.memset(sqg[:], 1.0)
                nc.vector.memset(skg[:], 1.0)
                nc.sync.dma_start(gg[:], cc_out.ap()[:, 0:224])
                nc.sync.dma_start(sqg[0:112, :], cc_out.ap()[:, 224:226])
                nc.sync.dma_start(skg[0:112, :], cc_out.ap()[:, 226:228])

                # ============ attention finalize ============
                def rsqrt_newton(tag, s_t):
                    sc = sbs.tile([128, 2], F32, tag=tag + "_c")
                    nc.vector.tensor_scalar_max(sc[:], s_t[:], 1e-24)
                    rt = sbs.tile([128, 2], F32, tag=tag + "_s")
                    nc.scalar.activation(rt[:], sc[:], AF.Sqrt)
                    r0 = sbs.tile([128, 2], F32, tag=tag + "_r0")
                    nc.vector.reciprocal(r0[:], rt[:])
                    rr = sbs.tile([128, 2], F32, tag=tag + "_rr")
                    nc.vector.tensor_tensor(out=rr[:], in0=r0[:], in1=r0[:], op=OP.mult)
                    t1_ = sbs.tile([128, 2], F32, tag=tag + "_t1")
                    nc.vector.scalar_tensor_tensor(out=t1_[:], in0=sc[:], scalar=-0.5,
                                                   in1=rr[:], op0=OP.mult, op1=OP.mult)
                    nc.vector.tensor_scalar_add(t1_[:], t1_[:], 1.5)
                    rv = sbs.tile([128, 2], F32, tag=tag)
                    nc.vector.tensor_tensor(out=rv[:], in0=r0[:], in1=t1_[:], op=OP.mult)
                    return rv

                rq = rsqrt_newton("rq", sqg)
                rk = rsqrt_newton("rk", skg)
                srow = sbs.tile([128, 2], F32, tag="srow", name="srow")
                nc.vector.tensor_tensor(out=srow[:], in0=rq[:], in1=tempb[:], op=OP.mult)

                srow_r, scol_r = [], []
                for p in range(2):
                    for src, lst, nm in ((srow, srow_r, "sr"), (rk, scol_r, "sc")):
                        fp = psm.tile([1, 112], F32, tag="sm", name="sm")
                        nc.tensor.transpose(fp[:], src[0:112, p:p + 1],
                                            id128[0:112, 0:112])
                        fr = sbs.tile([1, 112], F32R, tag=f"{nm}{p}", name=f"{nm}{p}")
                        nc.vector.tensor_copy(fr[:], fp[:])
                        lst.append(fr)

                attnT = [sbs.tile([112, 64], BF16, tag=f"attnT{p}", name=f"attnT{p}") for p in range(2)]
                for p in range(2):
                    nc.gpsimd.memset(attnT[p][:], 0.0)
                for p in range(2):
                    spair = psm.tile([112, 112], F32, tag="sm", name="sm")
                    nc.tensor.matmul(spair[:], srow_r[p][:], scol_r[p][:],
                                     start=True, stop=True)
                    lg = sbs.tile([112, 112], F32, tag="lg", name="lg")
                    nc.vector.tensor_tensor(out=lg[:], in0=gg[:, 112 * p:112 * (p + 1)],
                                            in1=spair[:], op=OP.mult)
                    at16 = sbs.tile([112, 112], BF16, tag="at16", name="at16")
                    for e in range(2):
                        sl = slice(64 * e, 64 * e + 48)
                        mx = sbs.tile([112, 1], F32, tag="mx", name="mx")
                        nc.vector.tensor_reduce(mx[sl, :], lg[sl, sl],
                                                axis=mybir.AxisListType.X, op=OP.max)
                        exh = sbs.tile([112, 112], F32, tag="exh", name="exh")
                        nc.vector.tensor_scalar(out=exh[sl, 0:48], in0=lg[sl, sl],
                                                scalar1=mx[sl, :], scalar2=None,
                                                op0=OP.subtract)
                        ex2 = sbs.tile([112, 112], F32, tag="ex2", name="ex2")
                        den = sbs.tile([112, 1], F32, tag="den", name="den")
                        nc.scalar.activation(ex2[sl, 0:48], exh[sl, 0:48], AF.Exp,
                                             accum_out=den[sl, :])
                        rc0 = sbs.tile([112, 1], F32, tag="rc0", name="rc0")
                        nc.vector.reciprocal(rc0[sl, :], den[sl, :])
                        nt = sbs.tile([112, 1], F32, tag="nt", name="nt")
                        nc.vector.tensor_tensor(out=nt[sl, :], in0=den[sl, :],
                                                in1=rc0[sl, :], op=OP.mult)
                        nc.vector.tensor_scalar(out=nt[sl, :], in0=nt[sl, :],
                                                scalar1=-1.0, scalar2=2.0,
                                                op0=OP.mult, op1=OP.add)
                        rc1 = sbs.tile([112, 1], F32, tag="rc1", name="rc1")
                        nc.vector.tensor_tensor(out=rc1[sl, :], in0=rc0[sl, :],
                                                in1=nt[sl, :], op=OP.mult)
                        nc.vector.tensor_scalar(out=at16[sl, 0:48], in0=ex2[sl, 0:48],
                                                scalar1=rc1[sl, :], scalar2=None,
                                                op0=OP.mult)
                        nc.sync.dma_start(d_attn[p, e], at16[sl, 0:48])
                        nc.sync.dma_start(
                            attnT[p][sl, 0:48],
                            d_attn[p, e].rearrange("a b -> b a"))

                conv_dw_path(d_ye, wkv_t, kvdw_t, 1, v_sink, m_off=3,
                             collect_mms=v_mms)
                for vm in v_mms:
                    for gl in gram_last:
                        _add_dep_helper(vm.ins, gl.ins, sync=False,
                                        reason="v after gram covers allreduce")

                if STAGE < 4:
                    ocx = sbs.tile([112, 64], F32, tag="ocx", name="ocx")
                    nc.vector.tensor_copy(ocx[:], attnT[0][:])
                    nc.sync.dma_start(d_out[0:112, 0:64], ocx[:])
                # ============ per-chunk trunk ============
                for ck in range(NCK if STAGE >= 4 else 0):
                    c0 = ck * 512
                    zp = [pdw.tile([128, 512], F32, tag="dw", name="dw") for _ in range(2)]
                    for p in range(2):
                        for e in range(2):
                            sl = slice(64 * e, 64 * e + 48)
                            osl = slice(64 * e, 64 * e + 64)
                            nc.tensor.matmul(zp[p][osl, :], attnT[p][sl, :],
                                             vband[p][sl, c0:c0 + 512],
                                             start=True, stop=True)
                    zc = [sbc.tile([128, 512], BF16, tag=f"zc{m}", name=f"zc{m}") for m in range(2)]
                    nc.vector.tensor_copy(zc[0][:], zp[0][:])
                    nc.scalar.copy(zc[1][:], zp[1][:])
                    tp = [pcv.tile([128, 512], F32, tag="cv", name="cv") for _ in range(2)]
                    for mi, (mo, ms) in enumerate(KB):
                        for k2 in range(2):
                            nc.tensor.matmul(tp[mi][:ms, :],
                                             wlin_t[k2][:, mo:mo + ms], zc[k2][:],
                                             start=(k2 == 0), stop=(k2 == 1))
                    ycn = sbc.tile([128, 2, 512], F32, tag="ycn", name="ycn")
                    for mi, (mo, ms) in enumerate(KB):
                        nc.sync.dma_start(ycn[:ms, mi, :], d_yc[mo:mo + ms, c0:c0 + 512])
                    t1c = [sbc.tile([s, 512], BF16, tag=f"t1c{i}", name=f"t1c{i}")
                           for i, (o, s) in enumerate(KB)]
                    for mi, (mo, ms) in enumerate(KB):
                        nc.vector.scalar_tensor_tensor(
                            out=t1c[mi][:], in0=ycn[:ms, mi, :], scalar=alphav[:ms, :],
                            in1=tp[mi][:ms, :], op0=OP.mult, op1=OP.add)
                    gc = [sbg.tile([128, 512], BF16, tag="gc", name="gc") for _ in range(6)]
                    for mt in range(6):
                        fp1 = pcv.tile([128, 512], F32, tag="cv", name="cv")
                        for i in range(2):
                            nc.tensor.matmul(fp1[:], wf1_t[i][:, 128 * mt:128 * (mt + 1)],
                                             t1c[i][:], start=(i == 0), stop=(i == 1))
                        nc.scalar.activation(gc[mt][:], fp1[:], AF.Gelu)
                    t2c = [sbc.tile([s, 512], BF16, tag=f"t2c{i}", name=f"t2c{i}")
                           for i, (o, s) in enumerate(KB)]
                    for mi, (mo, ms) in enumerate(KB):
                        fp2 = pcv.tile([128, 512], F32, tag="cv", name="cv")
                        for k in range(6):
                            nc.tensor.matmul(fp2[:ms, :], wf2_t[k][:, mo:mo + ms],
                                             gc[k][:], start=(k == 0), stop=(k == 5))
                        nc.vector.scalar_tensor_tensor(
                            out=t2c[mi][:], in0=t1c[mi][:], scalar=gammav[:ms, :],
                            in1=fp2[:ms, :], op0=OP.mult, op1=OP.add)
                    for mi, (mo, ms) in enumerate(KB):
                        pp = pcv.tile([128, 512], F32, tag="cv", name="cv")
                        for i in range(2):
                            nc.tensor.matmul(pp[:ms, :], wpr_t[i][:, mo:mo + ms],
                                             t2c[i][:], start=(i == 0), stop=(i == 1))
                        if DIRECT_PSUM_OUT:
                            nc.sync.dma_start(d_out[mo:mo + ms, c0:c0 + 512],
                                              pp[:ms, :])
                        else:
                            oc = sbc.tile([128, 512], F32, tag=f"oc{mi}", name=f"oc{mi}")
                            nc.scalar.copy(oc[:ms, :], pp[:ms, :])
                            nc.sync.dma_start(d_out[mo:mo + ms, c0:c0 + 512],
                                              oc[:ms, :])

    nc.compile()
    return nc


_NC = None


def _get_nc():
    global _NC
    if _NC is None:
        _NC = build_nc()
    return _NC


def _prep_weights(q_w, q_dw_w, kv_w, kv_dw_w, linear_w, proj_w, ffn1_w, ffn2_w,
                  temperature, alpha, beta, gamma, delta):
    def pad_oc(w):  # [192 real oc, ic] -> [ic, 256 padded oc]
        out = np.zeros((C, CP), np.float32)
        for h in range(HEADS):
            out[:, CPH * h:CPH * h + CH] = w[CH * h:CH * (h + 1), :].T
        return out

    wq = pad_oc(np.asarray(q_w, np.float32))
    kv = np.asarray(kv_w, np.float32)
    wkv = np.concatenate([pad_oc(kv[:C]), pad_oc(kv[C:])], axis=1)

    def pad_dw(w):  # [192,1,3,3] -> [256, 9, 32] diag blocks
        out = np.zeros((CP, 9, 32), np.float32)
        for h in range(HEADS):
            for j in range(CH):
                cp = CPH * h + j
                out[cp, :, cp % 32] = w[CH * h + j, 0].reshape(9)
        return out

    qdw = pad_dw(np.asarray(q_dw_w, np.float32))
    kvd = np.asarray(kv_dw_w, np.float32)
    kvdw = np.concatenate([pad_dw(kvd[:C]), pad_dw(kvd[C:])], axis=0)

    lin = np.asarray(linear_w, np.float32) * float(beta)
    wlin = np.zeros((CP, C), np.float32)
    for h in range(HEADS):
        wlin[CPH * h:CPH * h + CH, :] = lin[:, CH * h:CH * (h + 1)].T

    wf1 = np.asarray(ffn1_w, np.float32).T.copy()
    wf2 = (np.asarray(ffn2_w, np.float32) * float(delta)).T.copy()
    wpr = np.asarray(proj_w, np.float32).T.copy()

    tempb = np.zeros((128, 2), np.float32)
    tv = np.asarray(temperature, np.float32).reshape(HEADS)
    for h in range(HEADS):
        tempb[64 * (h % 2):64 * (h % 2) + 64, h // 2] = tv[h]

    alphav = np.full((128, 1), float(alpha), np.float32)
    gammav = np.full((128, 1), float(gamma), np.float32)
    id128 = np.eye(128, dtype=np.float32)
    idrep = np.zeros((128, 64), np.float32)
    for p_ in range(128):
        idrep[p_, p_ % 64] = 1.0

    return {
        "wq": wq.astype(bf16), "wkv": wkv.astype(bf16),
        "qdw": qdw.astype(bf16), "kvdw": kvdw.astype(bf16),
        "wlin": wlin.astype(bf16), "wf1": wf1.astype(bf16), "wf2": wf2.astype(bf16), "wpr": wpr.astype(bf16),
        "tempb": tempb, "alpha": alphav, "gamma": gammav,
        "id128": id128, "idrep": idrep,
    }


def kernel(**inputs):
    x = np.asarray(inputs["x"], np.float32)
    y = np.asarray(inputs["y"], np.float32)
    shared = _prep_weights(
        inputs["q_w"], inputs["q_dw_w"], inputs["kv_w"], inputs["kv_dw_w"],
        inputs["linear_w"], inputs["proj_w"], inputs["ffn1_w"], inputs["ffn2_w"],
        inputs["temperature"], inputs["alpha"], inputs["beta"],
        inputs["gamma"], inputs["delta"])

    in_maps = []
    for c in range(N_CORES):
        bi, s = c // 2, c % 2
        r0 = s * HLOC
        xe = np.zeros((C, ER, EC), np.float32)
        ye = np.zeros((C, ER, EC), np.float32)
        rlo, rhi = max(r0 - 1, 0), min(r0 + HLOC + 1, H)
        elo = rlo - (r0 - 1)
        xe[:, elo:elo + (rhi - rlo), 1:129] = x[bi, :, rlo:rhi, :]
        ye[:, elo:elo + (rhi - rlo), 1:129] = y[bi, :, rlo:rhi, :]
        m = dict(shared)
        m["xe"] = xe.reshape(C, NEXT).astype(bf16)
        m["ye"] = ye.reshape(C, NEXT).astype(bf16)
        m["yc"] = y[bi, :, r0:r0 + HLOC, :].reshape(C, NLOC).astype(np.float32)
        in_maps.append(m)

    nc = _get_nc()
    res = run_bass_kernel_spmd(nc, in_maps, list(range(N_CORES)))
    out = np.empty((B, C, H, W), np.float32)
    for c in range(N_CORES):
        bi, s = c // 2, c % 2
        out[bi, :, s * HLOC:(s + 1) * HLOC, :] = \
            res.results[c]["out"].reshape(C, HLOC, W)
    return out



# revision 8
# speedup vs baseline: 1.0693x; 1.0693x over previous
"""Trainium2 Bass kernel for nn_CDEM_62079457296798 (channel-attention
transformer block).

Sharding: 8 cores = 4 batches x 2 spatial halves (64 rows + 1 halo row each).
Cross-core communication: one small AllReduce per core-pair carrying the
channel-attention Gram matrices and q/k l2-norm sums; everything else local.

Layout: channel-major activations [C_part, pixels_free]. The attention path
(q/kv convs, depthwise 3x3, Gram, z) uses per-head channel padding 48 -> 64
(256 padded channels) so head boundaries are 32/64 aligned, and runs in bf16.
The depthwise 3x3 runs on the tensor engine as 9 accumulated diag-block
matmuls per 32-channel group with 16-way tile_position packing. The trunk
(linear/ffn/proj) runs in float32r (full-rate fp32).
"""
import sys
sys.path.insert(0, '/opt/trn_rl_repo')

import numpy as np
import ml_dtypes

import bass_rust
from concourse import bacc, mybir, tile
from concourse.bass import _add_dep_helper
from concourse.bass_utils import run_bass_kernel_spmd

F32 = mybir.dt.float32
F32R = mybir.dt.float32r
BF16 = mybir.dt.bfloat16
FP8 = mybir.dt.float8e4
DRM = mybir.MatmulPerfMode.DoubleRow
AF = mybir.ActivationFunctionType
OP = mybir.AluOpType
bf16 = ml_dtypes.bfloat16
f8 = ml_dtypes.float8_e4m3fn

# depthwise 3x3 as 4 fp8 DoubleRow pairs + 1 single (tap index t = 3*(dr+1)+(dc+1));
# pair strides in elements of the [ER, EC] image (2 = two cols, 260 = two rows)
DW_PAIRS = [(0, 2, 2), (3, 5, 2), (6, 8, 2), (1, 7, 2 * 130)]
DW_SINGLE = 4


def _pair_ap(base, stride):
    raw = base.ap.copy()
    return bass_rust.AP(base.tensor, base.offset,
                        [raw[0], [stride, 2]] + list(raw[1:]))

N_CORES = 8
B, C, H, W = 4, 192, 128, 128
HEADS, CH = 4, 48
CPH = 64                # padded channels per head
CP = HEADS * CPH        # 256 padded attn channels
HLOC = 64               # image rows per core
ER, EC = 66, 130        # ext rows/cols (halo + zero pad)
NEXT = ER * EC          # 8580
NLOC = HLOC * W         # 8192
NCK = 16                # output chunks (4 rows x 128 = 512 px)
CONV_CHUNKS = [(i * 512, 512) for i in range(16)] + [(16 * 512, NEXT - 16 * 512)]
GRP = 2048
CONV_GROUPS = [(i * GRP, GRP) for i in range(4)] + [(4 * GRP, NEXT - 4 * GRP)]
KB = [(0, 128), (128, 64)]          # 192-channel K bands

DIRECT_PSUM_OUT = False  # DMA final result straight from PSUM


import os
STAGE = int(os.environ.get("KSTAGE", "4"))
KSUB = int(os.environ.get("KSUB", "4"))


class _StageDone(Exception):
    pass


def build_nc():
    nc = bacc.Bacc("TRN2", target_bir_lowering=False, debug=False,
                   num_devices=N_CORES)

    d_xe = nc.dram_tensor("xe", [C, NEXT], BF16, kind="ExternalInput")
    d_ye = nc.dram_tensor("ye", [C, NEXT], BF16, kind="ExternalInput")
    d_yc = nc.dram_tensor("yc", [C, NLOC], F32, kind="ExternalInput")
    d_wq = nc.dram_tensor("wq", [C, CP], BF16, kind="ExternalInput")
    d_wkv = nc.dram_tensor("wkv", [C, 2 * CP], BF16, kind="ExternalInput")
    d_qdw = nc.dram_tensor("qdw", [CP, 9, 128], FP8, kind="ExternalInput")
    d_kvdw = nc.dram_tensor("kvdw", [2 * CP, 9, 128], FP8, kind="ExternalInput")
    d_wlin = nc.dram_tensor("wlin", [CP, C], BF16, kind="ExternalInput")
    d_wf1 = nc.dram_tensor("wf1", [C, 768], BF16, kind="ExternalInput")
    d_wf2 = nc.dram_tensor("wf2", [768, C], BF16, kind="ExternalInput")
    d_wpr = nc.dram_tensor("wpr", [C, C], BF16, kind="ExternalInput")
    d_tempb = nc.dram_tensor("tempb", [128, 2], F32, kind="ExternalInput")
    d_alpha = nc.dram_tensor("alpha", [128, 1], F32, kind="ExternalInput")
    d_gamma = nc.dram_tensor("gamma", [128, 1], F32, kind="ExternalInput")
    d_id128 = nc.dram_tensor("id128", [128, 128], F32, kind="ExternalInput")
    d_idrep = nc.dram_tensor("idrep", [128, 64], F32, kind="ExternalInput")
    d_out = nc.dram_tensor("out", [C, NLOC], F32, kind="ExternalOutput")
    d_attn = nc.dram_tensor("attn_bounce", [2, 2, 48, 48], BF16)
    cc_in = nc.dram_tensor("cc_in", [112, 228], F32)
    cc_out = nc.dram_tensor("cc_out", [112, 228], F32)

    with tile.TileContext(nc) as tc:
        with (
            tc.tile_pool(name="sbw", bufs=1) as sbw,      # weights/consts
            tc.tile_pool(name="sbpre", bufs=1) as sbpre,  # conv1x1 out (ext img)
            tc.tile_pool(name="sbin", bufs=3) as sbin,    # streamed conv inputs
            tc.tile_pool(name="sbqk", bufs=4) as sbqk,    # q/k chunk tiles
            tc.tile_pool(name="sbT", bufs=1) as sbT,      # qT/kT/v persistents
            tc.tile_pool(name="sbs", bufs=1) as sbs,      # small attn tiles
            tc.tile_pool(name="sbc", bufs=2) as sbc,      # trunk chunk pipeline
            tc.tile_pool(name="sbg", bufs=6) as sbg,      # gelu chunk tiles
            tc.tile_pool(name="pcv", bufs=4, space="PSUM") as pcv,
            tc.tile_pool(name="pdw", bufs=2, space="PSUM") as pdw,
            tc.tile_pool(name="pacc", bufs=1, space="PSUM") as pacc,
            tc.tile_pool(name="psm", bufs=1, space="PSUM") as psm,
        ):
            # ---------- weights ----------
            wq_t = [sbw.tile([s, CP], BF16, tag=f"wq{i}", name=f"wq{i}")
                    for i, (o, s) in enumerate(KB)]
            wkv_t = [sbw.tile([s, 2 * CP], BF16, tag=f"wkv{i}", name=f"wkv{i}")
                     for i, (o, s) in enumerate(KB)]
            for i, (o, s) in enumerate(KB):
                nc.sync.dma_start(wq_t[i][:], d_wq[o:o + s, :])
            qdw_t = [sbw.tile([128, 9, 128], FP8, tag=f"qdw{m}", name=f"qdw{m}") for m in range(2)]
            kvdw_t = [sbw.tile([128, 9, 128], FP8, tag=f"kvdw{m}", name=f"kvdw{m}") for m in range(4)]
            for m in range(2):
                nc.sync.dma_start(qdw_t[m][:], d_qdw[128 * m:128 * (m + 1)])

            def load_kv_weights():
                for i, (o, s) in enumerate(KB):
                    nc.sync.dma_start(wkv_t[i][:], d_wkv[o:o + s, :])
                for m in range(4):
                    nc.sync.dma_start(kvdw_t[m][:], d_kvdw[128 * m:128 * (m + 1)])
            wlin_t = [sbw.tile([128, C], BF16, tag=f"wlin{m}", name=f"wlin{m}") for m in range(2)]
            wf1_t = [sbw.tile([s, 768], BF16, tag=f"wf1{i}", name=f"wf1{i}")
                     for i, (o, s) in enumerate(KB)]
            wf2_t = [sbw.tile([128, C], BF16, tag=f"wf2{k}", name=f"wf2{k}") for k in range(6)]
            wpr_t = [sbw.tile([s, C], BF16, tag=f"wpr{i}", name=f"wpr{i}")
                     for i, (o, s) in enumerate(KB)]
            tempb = sbw.tile([128, 2], F32, tag="tempb", name="tempb")
            alphav = sbw.tile([128, 1], F32, tag="alphav", name="alphav")
            gammav = sbw.tile([128, 1], F32, tag="gammav", name="gammav")
            id128 = sbw.tile([128, 128], F32, tag="id128", name="id128")
            idrep = sbw.tile([128, 64], F32, tag="idrep", name="idrep")

            def load_trunk_weights():
                for m in range(2):
                    nc.sync.dma_start(wlin_t[m][:], d_wlin[128 * m:128 * (m + 1), :])
                for i, (o, s) in enumerate(KB):
                    nc.sync.dma_start(wf1_t[i][:], d_wf1[o:o + s, :])
                for k in range(6):
                    nc.sync.dma_start(wf2_t[k][:], d_wf2[128 * k:128 * (k + 1), :])
                for i, (o, s) in enumerate(KB):
                    nc.sync.dma_start(wpr_t[i][:], d_wpr[o:o + s, :])
                nc.sync.dma_start(tempb[:], d_tempb.ap())
                nc.sync.dma_start(alphav[:], d_alpha.ap())
                nc.sync.dma_start(gammav[:], d_gamma.ap())
                nc.sync.dma_start(id128[:], d_id128.ap())
                nc.sync.dma_start(idrep[:], d_idrep.ap())

            # persistent attn-path results
            qT = [sbT.tile([128, 64, 112], BF16, tag=f"qT{p}", name=f"qT{p}") for p in range(2)]
            kT = [sbT.tile([128, 64, 112], BF16, tag=f"kT{p}", name=f"kT{p}") for p in range(2)]
            vband = [sbT.tile([128, NLOC], BF16, tag=f"v{m}", name=f"v{m}") for m in range(2)]
            sqp = [sbs.tile([128, NCK], F32, tag=f"sqp{i}", name=f"sqp{i}") for i in range(4)]
            for i in range(4):
                nc.vector.memset(sqp[i][:], 0.0)
            gacc = pacc.tile([112, 224], F32, tag="gacc", name="gacc")

            # ============ q/k/v production ============
            def conv_dw_path(src_dram, w_t, dw_tiles, n_mb, sink, m_off=0,
                             collect_mms=None):
                """192 -> n_mb*128 padded conv1x1 + depthwise 3x3 per band.
                sink(m, ck, psum_flat_ap) consumes each dwconv chunk."""
                for m in range(m_off, m_off + n_mb):
                    pre = sbpre.tile([128, ER, EC], FP8, tag="pre", name="pre")
                    pref = pre[:].rearrange("p a b -> p (a b)")
                    ci = 0
                    for g0, gn in CONV_GROUPS:
                        xc = [sbin.tile([s, GRP], BF16, tag=f"xin{i}", name=f"xin{i}")
                              for i, (o, s) in enumerate(KB)]
                        for i, (o, s) in enumerate(KB):
                            nc.sync.dma_start(xc[i][:, :gn],
                                              src_dram[o:o + s, g0:g0 + gn])
                        for c0 in range(0, gn, 512):
                            cn = min(512, gn - c0)
                            ps = pcv.tile([128, 512], F32, tag="cv", name="cv")
                            for i in range(2):
                                mm = nc.tensor.matmul(
                                    ps[:, :cn],
                                    w_t[i][:, 128 * m:128 * (m + 1)],
                                    xc[i][:, c0:c0 + cn],
                                    start=(i == 0), stop=(i == 1))
                                if collect_mms is not None:
                                    collect_mms.append(mm)
                            if ci % 2 == 0:
                                nc.vector.tensor_copy(pref[:, g0 + c0:g0 + c0 + cn],
                                                      ps[:, :cn])
                            else:
                                nc.scalar.copy(pref[:, g0 + c0:g0 + c0 + cn],
                                               ps[:, :cn])
                            ci += 1
                    for ck in range(NCK if KSUB >= 2 else 0):
                        r0 = 1 + 4 * ck
                        dp = pdw.tile([128, 4, 128], F32, tag="dw", name="dw")
                        for i, (ta, tb, stride) in enumerate(DW_PAIRS):
                            dra, dca = ta // 3 - 1, ta % 3 - 1
                            base = pre[:, r0 + dra:r0 + 4 + dra,
                                       1 + dca:129 + dca]
                            nc.tensor.matmul(
                                dp[:], dw_tiles[m][:, 2 * i:2 * i + 2, :],
                                _pair_ap(base, stride),
                                start=(i == 0), stop=False, perf_mode=DRM)
                        nc.tensor.matmul(
                            dp[:], dw_tiles[m][:, 8, :],
                            pre[:, r0:r0 + 4, 1:129],
                            start=False, stop=True)
                        sink(m, ck, dp[:].rearrange("p a b -> p (a b)"))

            def qk_sink(dstT, sq_idx):
                qcbig = [None]

                def sink(m, ck, flat):
                    j = ck % 4
                    if j == 0:
                        qcbig[0] = sbqk.tile([128, 2048], BF16, tag="qkc", name="qkc")
                    qc = qcbig[0][:, 512 * j:512 * (j + 1)]
                    if ck % 2 == 0:
                        nc.vector.tensor_copy(qc, flat)
                    else:
                        nc.scalar.copy(qc, flat)
                    if KSUB >= 3:
                        dmp = sbqk.tile([128, 512], F32, tag="dump", name="dump")
                        nc.scalar.activation(dmp[:], qc, AF.Square,
                                             accum_out=sqp[sq_idx + m][:, ck:ck + 1])
                    if KSUB >= 4 and j == 3:
                        nc.sync.dma_start_transpose(
                            dstT[m][:, 4 * ck - 12:4 * ck + 4, :],
                            qcbig[0][0:112, :])
                return sink

            load_kv_weights()
            conv_dw_path(d_xe, wq_t, qdw_t, 2, qk_sink(qT, 0))

            if STAGE >= 2:

                def v_sink(m, ck, flat):
                    dst = vband[m - 2]
                    if ck % 2 == 0:
                        nc.vector.tensor_copy(dst[:, ck * 512:(ck + 1) * 512], flat)
                    else:
                        nc.scalar.copy(dst[:, ck * 512:(ck + 1) * 512], flat)

                conv_dw_path(d_ye, wkv_t, kvdw_t, 2, qk_sink(kT, 2))
                load_trunk_weights()

            # ============ Gram accumulation ============
            gram_last = []
            for p in range(STAGE >= 2 and 2 or 0):
                for ck in range(64):
                    gmm = nc.tensor.matmul(gacc[:, 112 * p:112 * (p + 1)],
                                           qT[p][:, ck, :], kT[p][:, ck, :],
                                           start=(ck == 0), stop=(ck == 63))
                    if ck == 63:
                        gram_last.append(gmm)

            if STAGE < 3:
                oc0 = sbs.tile([128, NCK], F32, tag="oc0d", name="oc0d")
                nc.vector.tensor_copy(oc0[:], sqp[0][:])
                nc.sync.dma_start(d_out[0:128, 0:NCK], oc0[:])
            if STAGE >= 3:
                sqv = sbs.tile([128, 2], F32, tag="sqv", name="sqv")
                skv = sbs.tile([128, 2], F32, tag="skv", name="skv")
                for m in range(2):
                    nc.vector.tensor_reduce(sqv[:, m:m + 1], sqp[m][:],
                                            axis=mybir.AxisListType.X, op=OP.add)
                    nc.vector.tensor_reduce(skv[:, m:m + 1], sqp[2 + m][:],
                                            axis=mybir.AxisListType.X, op=OP.add)

                # ============ pair AllReduce ============
                gsb = sbs.tile([112, 224], F32, tag="gsb", name="gsb")
                nc.vector.tensor_copy(gsb[:], gacc[:])
                nc.scalar.dma_start(cc_in.ap()[:, 0:224], gsb[:])
                nc.scalar.dma_start(cc_in.ap()[:, 224:226], sqv[0:112, :])
                nc.scalar.dma_start(cc_in.ap()[:, 226:228], skv[0:112, :])
                nc.gpsimd.collective_compute(
                    "AllReduce", OP.add,
                    replica_groups=[[0, 1], [2, 3], [4, 5], [6, 7]],
                    ins=[cc_in.ap()], outs=[cc_out.ap()])
                # v(m2) here so it overlaps the AllReduce + softmax
                v_mms = []
                conv_dw_path(d_ye, wkv_t, kvdw_t, 1, v_sink, m_off=2,
                             collect_mms=v_mms)
                gg = sbs.tile([112, 224], F32, tag="gg", name="gg")
                sqg = sbs.tile([128, 2], F32, tag="sqg", name="sqg")
                skg = sbs.tile([128, 2], F32, tag="skg", name="skg")
                nc.vector.memset(sqg[:], 1.0)
                nc.vector.memset(skg[:], 1.0)
                nc.sync.dma_start(gg[:], cc_out.ap()[:, 0:224])
                nc.sync.dma_start(sqg[0:112, :], cc_out.ap()[:, 224:226])
                nc.sync.dma_start(skg[0:112, :], cc_out.ap()[:, 226:228])

                # ============ attention finalize ============
                def rsqrt_newton(tag, s_t):
                    sc = sbs.tile([128, 2], F32, tag=tag + "_c")
                    nc.vector.tensor_scalar_max(sc[:], s_t[:], 1e-24)
                    rt = sbs.tile([128, 2], F32, tag=tag + "_s")
                    nc.scalar.activation(rt[:], sc[:], AF.Sqrt)
                    r0 = sbs.tile([128, 2], F32, tag=tag + "_r0")
                    nc.vector.reciprocal(r0[:], rt[:])
                    rr = sbs.tile([128, 2], F32, tag=tag + "_rr")
                    nc.vector.tensor_tensor(out=rr[:], in0=r0[:], in1=r0[:], op=OP.mult)
                    t1_ = sbs.tile([128, 2], F32, tag=tag + "_t1")
                    nc.vector.scalar_tensor_tensor(out=t1_[:], in0=sc[:], scalar=-0.5,
                                                   in1=rr[:], op0=OP.mult, op1=OP.mult)
                    nc.vector.tensor_scalar_add(t1_[:], t1_[:], 1.5)
                    rv = sbs.tile([128, 2], F32, tag=tag)
                    nc.vector.tensor_tensor(out=rv[:], in0=r0[:], in1=t1_[:], op=OP.mult)
                    return rv

                rq = rsqrt_newton("rq", sqg)
                rk = rsqrt_newton("rk", skg)
                srow = sbs.tile([128, 2], F32, tag="srow", name="srow")
                nc.vector.tensor_tensor(out=srow[:], in0=rq[:], in1=tempb[:], op=OP.mult)

                srow_r, scol_r = [], []
                for p in range(2):
                    for src, lst, nm in ((srow, srow_r, "sr"), (rk, scol_r, "sc")):
                        fp = psm.tile([1, 112], F32, tag="sm", name="sm")
                        nc.tensor.transpose(fp[:], src[0:112, p:p + 1],
                                            id128[0:112, 0:112])
                        fr = sbs.tile([1, 112], F32R, tag=f"{nm}{p}", name=f"{nm}{p}")
                        nc.vector.tensor_copy(fr[:], fp[:])
                        lst.append(fr)

                attnT = [sbs.tile([112, 64], BF16, tag=f"attnT{p}", name=f"attnT{p}") for p in range(2)]
                for p in range(2):
                    nc.gpsimd.memset(attnT[p][:], 0.0)
                for p in range(2):
                    spair = psm.tile([112, 112], F32, tag="sm", name="sm")
                    nc.tensor.matmul(spair[:], srow_r[p][:], scol_r[p][:],
                                     start=True, stop=True)
                    lg = sbs.tile([112, 112], F32, tag="lg", name="lg")
                    nc.vector.tensor_tensor(out=lg[:], in0=gg[:, 112 * p:112 * (p + 1)],
                                            in1=spair[:], op=OP.mult)
                    at16 = sbs.tile([112, 112], BF16, tag="at16", name="at16")
                    for e in range(2):
                        sl = slice(64 * e, 64 * e + 48)
                        mx = sbs.tile([112, 1], F32, tag="mx", name="mx")
                        nc.vector.tensor_reduce(mx[sl, :], lg[sl, sl],
                                                axis=mybir.AxisListType.X, op=OP.max)
                        exh = sbs.tile([112, 112], F32, tag="exh", name="exh")
                        nc.vector.tensor_scalar(out=exh[sl, 0:48], in0=lg[sl, sl],
                                                scalar1=mx[sl, :], scalar2=None,
                                                op0=OP.subtract)
                        ex2 = sbs.tile([112, 112], F32, tag="ex2", name="ex2")
                        den = sbs.tile([112, 1], F32, tag="den", name="den")
                        nc.scalar.activation(ex2[sl, 0:48], exh[sl, 0:48], AF.Exp,
                                             accum_out=den[sl, :])
                        rc0 = sbs.tile([112, 1], F32, tag="rc0", name="rc0")
                        nc.vector.reciprocal(rc0[sl, :], den[sl, :])
                        nt = sbs.tile([112, 1], F32, tag="nt", name="nt")
                        nc.vector.tensor_tensor(out=nt[sl, :], in0=den[sl, :],
                                                in1=rc0[sl, :], op=OP.mult)
                        nc.vector.tensor_scalar(out=nt[sl, :], in0=nt[sl, :],
                                                scalar1=-1.0, scalar2=2.0,
                                                op0=OP.mult, op1=OP.add)
                        rc1 = sbs.tile([112, 1], F32, tag="rc1", name="rc1")
                        nc.vector.tensor_tensor(out=rc1[sl, :], in0=rc0[sl, :],
                                                in1=nt[sl, :], op=OP.mult)
                        nc.vector.tensor_scalar(out=at16[sl, 0:48], in0=ex2[sl, 0:48],
                                                scalar1=rc1[sl, :], scalar2=None,
                                                op0=OP.mult)
                        nc.sync.dma_start(d_attn[p, e], at16[sl, 0:48])
                        nc.sync.dma_start(
                            attnT[p][sl, 0:48],
                            d_attn[p, e].rearrange("a b -> b a"))

                conv_dw_path(d_ye, wkv_t, kvdw_t, 1, v_sink, m_off=3,
                             collect_mms=v_mms)
                for vm in v_mms:
                    for gl in gram_last:
                        _add_dep_helper(vm.ins, gl.ins, sync=False,
                                        reason="v after gram covers allreduce")

                if STAGE < 4:
                    ocx = sbs.tile([112, 64], F32, tag="ocx", name="ocx")
                    nc.vector.tensor_copy(ocx[:], attnT[0][:])
                    nc.sync.dma_start(d_out[0:112, 0:64], ocx[:])
                # ============ per-chunk trunk ============
                for ck in range(NCK if STAGE >= 4 else 0):
                    c0 = ck * 512
                    zp = [pdw.tile([128, 512], F32, tag="dw", name="dw") for _ in range(2)]
                    for p in range(2):
                        for e in range(2):
                            sl = slice(64 * e, 64 * e + 48)
                            osl = slice(64 * e, 64 * e + 64)
                            nc.tensor.matmul(zp[p][osl, :], attnT[p][sl, :],
                                             vband[p][sl, c0:c0 + 512],
                                             start=True, stop=True)
                    zc = [sbc.tile([128, 512], BF16, tag=f"zc{m}", name=f"zc{m}") for m in range(2)]
                    nc.vector.tensor_copy(zc[0][:], zp[0][:])
                    nc.scalar.copy(zc[1][:], zp[1][:])
                    tp = [pcv.tile([128, 512], F32, tag="cv", name="cv") for _ in range(2)]
                    for mi, (mo, ms) in enumerate(KB):
                        for k2 in range(2):
                            nc.tensor.matmul(tp[mi][:ms, :],
                                             wlin_t[k2][:, mo:mo + ms], zc[k2][:],
                                             start=(k2 == 0), stop=(k2 == 1))
                    ycn = sbc.tile([128, 2, 512], F32, tag="ycn", name="ycn")
                    for mi, (mo, ms) in enumerate(KB):
                        nc.sync.dma_start(ycn[:ms, mi, :], d_yc[mo:mo + ms, c0:c0 + 512])
                    t1c = [sbc.tile([s, 512], BF16, tag=f"t1c{i}", name=f"t1c{i}")
                           for i, (o, s) in enumerate(KB)]
                    for mi, (mo, ms) in enumerate(KB):
                        nc.vector.scalar_tensor_tensor(
                            out=t1c[mi][:], in0=ycn[:ms, mi, :], scalar=alphav[:ms, :],
                            in1=tp[mi][:ms, :], op0=OP.mult, op1=OP.add)
                    gc = [sbg.tile([128, 512], BF16, tag="gc", name="gc") for _ in range(6)]
                    for mt in range(6):
                        fp1 = pcv.tile([128, 512], F32, tag="cv", name="cv")
                        for i in range(2):
                            nc.tensor.matmul(fp1[:], wf1_t[i][:, 128 * mt:128 * (mt + 1)],
                                             t1c[i][:], start=(i == 0), stop=(i == 1))
                        nc.scalar.activation(gc[mt][:], fp1[:], AF.Gelu)
                    t2c = [sbc.tile([s, 512], BF16, tag=f"t2c{i}", name=f"t2c{i}")
                           for i, (o, s) in enumerate(KB)]
                    for mi, (mo, ms) in enumerate(KB):
                        fp2 = pcv.tile([128, 512], F32, tag="cv", name="cv")
                        for k in range(6):
                            nc.tensor.matmul(fp2[:ms, :], wf2_t[k][:, mo:mo + ms],
                                             gc[k][:], start=(k == 0), stop=(k == 5))
                        nc.vector.scalar_tensor_tensor(
                            out=t2c[mi][:], in0=t1c[mi][:], scalar=gammav[:ms, :],
                            in1=fp2[:ms, :], op0=OP.mult, op1=OP.add)
                    for mi, (mo, ms) in enumerate(KB):
                        pp = pcv.tile([128, 512], F32, tag="cv", name="cv")
                        for i in range(2):
                            nc.tensor.matmul(pp[:ms, :], wpr_t[i][:, mo:mo + ms],
                                             t2c[i][:], start=(i == 0), stop=(i == 1))
                        if DIRECT_PSUM_OUT:
                            nc.sync.dma_start(d_out[mo:mo + ms, c0:c0 + 512],
                                              pp[:ms, :])
                        else:
                            oc = sbc.tile([128, 512], F32, tag=f"oc{mi}", name=f"oc{mi}")
                            nc.scalar.copy(oc[:ms, :], pp[:ms, :])
                            nc.sync.dma_start(d_out[mo:mo + ms, c0:c0 + 512],
                                              oc[:ms, :])

    nc.compile()
    return nc


_NC = None


def _get_nc():
    global _NC
    if _NC is None:
        _NC = build_nc()
    return _NC


def _prep_weights(q_w, q_dw_w, kv_w, kv_dw_w, linear_w, proj_w, ffn1_w, ffn2_w,
                  temperature, alpha, beta, gamma, delta):
    def pad_oc(w):  # [192 real oc, ic] -> [ic, 256 padded oc]
        out = np.zeros((C, CP), np.float32)
        for h in range(HEADS):
            out[:, CPH * h:CPH * h + CH] = w[CH * h:CH * (h + 1), :].T
        return out

    wq = pad_oc(np.asarray(q_w, np.float32)) * 8.0
    kv = np.asarray(kv_w, np.float32)
    wkv = np.concatenate([pad_oc(kv[:C]), pad_oc(kv[C:])], axis=1) * 8.0

    # [192,1,3,3] -> [256, 9, 128] diag, slots = DW_PAIRS order + center
    slot_tap = [0, 2, 3, 5, 6, 8, 1, 7, 4]

    def pad_dw(w):
        out = np.zeros((CP, 9, 128), np.float32)
        for h in range(HEADS):
            for j in range(CH):
                cp = CPH * h + j
                taps = w[CH * h + j, 0].reshape(9)
                for s, t in enumerate(slot_tap):
                    out[cp, s, cp % 128] = taps[t]
        return out * 32.0

    qdw = pad_dw(np.asarray(q_dw_w, np.float32))
    kvd = np.asarray(kv_dw_w, np.float32)
    kvdw = np.concatenate([pad_dw(kvd[:C]), pad_dw(kvd[C:])], axis=0)

    lin = np.asarray(linear_w, np.float32) * (float(beta) / 256.0)
    wlin = np.zeros((CP, C), np.float32)
    for h in range(HEADS):
        wlin[CPH * h:CPH * h + CH, :] = lin[:, CH * h:CH * (h + 1)].T

    wf1 = np.asarray(ffn1_w, np.float32).T.copy()
    wf2 = (np.asarray(ffn2_w, np.float32) * float(delta)).T.copy()
    wpr = np.asarray(proj_w, np.float32).T.copy()

    tempb = np.zeros((128, 2), np.float32)
    tv = np.asarray(temperature, np.float32).reshape(HEADS)
    for h in range(HEADS):
        tempb[64 * (h % 2):64 * (h % 2) + 64, h // 2] = tv[h]

    alphav = np.full((128, 1), float(alpha), np.float32)
    gammav = np.full((128, 1), float(gamma), np.float32)
    id128 = np.eye(128, dtype=np.float32)
    idrep = np.zeros((128, 64), np.float32)
    for p_ in range(128):
        idrep[p_, p_ % 64] = 1.0

    return {
        "wq": wq.astype(bf16), "wkv": wkv.astype(bf16),
        "qdw": qdw.astype(f8), "kvdw": kvdw.astype(f8),
        "wlin": wlin.astype(bf16), "wf1": wf1.astype(bf16), "wf2": wf2.astype(bf16), "wpr": wpr.astype(bf16),
        "tempb": tempb, "alpha": alphav, "gamma": gammav,
        "id128": id128, "idrep": idrep,
    }


def kernel(**inputs):
    x = np.asarray(inputs["x"], np.float32)
    y = np.asarray(inputs["y"], np.float32)
    shared = _prep_weights(
        inputs["q_w"], inputs["q_dw_w"], inputs["kv_w"], inputs["kv_dw_w"],
        inputs["linear_w"], inputs["proj_w"], inputs["ffn1_w"], inputs["ffn2_w"],
        inputs["temperature"], inputs["alpha"], inputs["beta"],
        inputs["gamma"], inputs["delta"])

    in_maps = []
    for c in range(N_CORES):
        bi, s = c // 2, c % 2
        r0 = s * HLOC
        xe = np.zeros((C, ER, EC), np.float32)
        ye = np.zeros((C, ER, EC), np.float32)
        rlo, rhi = max(r0 - 1, 0), min(r0 + HLOC + 1, H)
        elo = rlo - (r0 - 1)
        xe[:, elo:elo + (rhi - rlo), 1:129] = x[bi, :, rlo:rhi, :]
        ye[:, elo:elo + (rhi - rlo), 1:129] = y[bi, :, rlo:rhi, :]
        m = dict(shared)
        m["xe"] = xe.reshape(C, NEXT).astype(bf16)
        m["ye"] = ye.reshape(C, NEXT).astype(bf16)
        m["yc"] = y[bi, :, r0:r0 + HLOC, :].reshape(C, NLOC).astype(np.float32)
        in_maps.append(m)

    nc = _get_nc()
    res = run_bass_kernel_spmd(nc, in_maps, list(range(N_CORES)))
    out = np.empty((B, C, H, W), np.float32)
    for c in range(N_CORES):
        bi, s = c // 2, c % 2
        out[bi, :, s * HLOC:(s + 1) * HLOC, :] = \
            res.results[c]["out"].reshape(C, HLOC, W)
    return out



# revision 13
# speedup vs baseline: 1.1657x; 1.0902x over previous
"""Trainium2 Bass kernel for nn_CDEM_62079457296798 (channel-attention
transformer block).

Sharding: 8 cores = 4 batches x 2 spatial halves (64 rows + 1 halo row each).
Cross-core communication: one small AllReduce per core-pair carrying the
channel-attention Gram matrices and q/k l2-norm sums; everything else local.

Layout: channel-major activations [C_part, pixels_free]. The attention path
(q/kv convs, depthwise 3x3, Gram, z) uses per-head channel padding 48 -> 64
(256 padded channels) so head boundaries are 32/64 aligned, and runs in bf16.
The depthwise 3x3 runs on the tensor engine as 9 accumulated diag-block
matmuls per 32-channel group with 16-way tile_position packing. The trunk
(linear/ffn/proj) runs in float32r (full-rate fp32).
"""
import sys
sys.path.insert(0, '/opt/trn_rl_repo')

import numpy as np
import ml_dtypes

import bass_rust
from concourse import bacc, mybir, tile
from concourse.bass import _add_dep_helper
from concourse.bass_utils import run_bass_kernel_spmd

F32 = mybir.dt.float32
F32R = mybir.dt.float32r
BF16 = mybir.dt.bfloat16
FP8 = mybir.dt.float8e4
DRM = mybir.MatmulPerfMode.DoubleRow
AF = mybir.ActivationFunctionType
OP = mybir.AluOpType
bf16 = ml_dtypes.bfloat16
f8 = ml_dtypes.float8_e4m3fn

# depthwise 3x3 as 4 fp8 DoubleRow pairs + 1 single (tap index t = 3*(dr+1)+(dc+1));
# pair strides in elements of the [ER, EC] image (2 = two cols, 260 = two rows)
DW_PAIRS = [(0, 2, 2), (3, 5, 2), (6, 8, 2), (1, 7, 2 * 130)]
DW_SINGLE = 4


def _pair_ap(base, stride):
    raw = base.ap.copy()
    return bass_rust.AP(base.tensor, base.offset,
                        [raw[0], [stride, 2]] + list(raw[1:]))

N_CORES = 8
B, C, H, W = 4, 192, 128, 128
HEADS, CH = 4, 48
CPH = 64                # padded channels per head
CP = HEADS * CPH        # 256 padded attn channels
HLOC = 64               # image rows per core
ER, EC = 66, 130        # ext rows/cols (halo + zero pad)
NEXT = ER * EC          # 8580
NLOC = HLOC * W         # 8192
NCK = 16                # output chunks (4 rows x 128 = 512 px)
CONV_CHUNKS = [(i * 512, 512) for i in range(16)] + [(16 * 512, NEXT - 16 * 512)]
GRP = 2048
CONV_GROUPS = [(i * GRP, GRP) for i in range(4)] + [(4 * GRP, NEXT - 4 * GRP)]
KB = [(0, 128), (128, 64)]          # 192-channel K bands

DIRECT_PSUM_OUT = False  # DMA final result straight from PSUM


import os
STAGE = int(os.environ.get("KSTAGE", "4"))
KSUB = int(os.environ.get("KSUB", "4"))


class _StageDone(Exception):
    pass


def build_nc():
    nc = bacc.Bacc("TRN2", target_bir_lowering=False, debug=False,
                   num_devices=N_CORES)

    d_xe = nc.dram_tensor("xe", [96, 2, NEXT], FP8, kind="ExternalInput")
    d_ye = nc.dram_tensor("ye", [96, 2, NEXT], FP8, kind="ExternalInput")
    d_yc = nc.dram_tensor("yc", [C, NLOC], F32, kind="ExternalInput")
    d_wq = nc.dram_tensor("wq", [96, 2, CP], FP8, kind="ExternalInput")
    d_wkv = nc.dram_tensor("wkv", [96, 2, 2 * CP], FP8, kind="ExternalInput")
    d_qdw = nc.dram_tensor("qdw", [CP, 9, 128], FP8, kind="ExternalInput")
    d_kvdw = nc.dram_tensor("kvdw", [2 * CP, 9, 128], FP8, kind="ExternalInput")
    d_wlin = nc.dram_tensor("wlin", [CP, C], BF16, kind="ExternalInput")
    d_wf1 = nc.dram_tensor("wf1", [C, 768], BF16, kind="ExternalInput")
    d_wf2 = nc.dram_tensor("wf2", [768, C], BF16, kind="ExternalInput")
    d_wpr = nc.dram_tensor("wpr", [C, C], BF16, kind="ExternalInput")
    d_tempb = nc.dram_tensor("tempb", [128, 2], F32, kind="ExternalInput")
    d_alpha = nc.dram_tensor("alpha", [128, 1], F32, kind="ExternalInput")
    d_gamma = nc.dram_tensor("gamma", [128, 1], F32, kind="ExternalInput")
    d_id128 = nc.dram_tensor("id128", [128, 128], F32, kind="ExternalInput")
    d_idrep = nc.dram_tensor("idrep", [128, 64], F32, kind="ExternalInput")
    d_out = nc.dram_tensor("out", [C, NLOC], F32, kind="ExternalOutput")
    d_attn = nc.dram_tensor("attn_bounce", [2, 2, 48, 48], BF16)
    cc_in = nc.dram_tensor("cc_in", [112, 228], F32)
    cc_out = nc.dram_tensor("cc_out", [112, 228], F32)

    with tile.TileContext(nc) as tc:
        with (
            tc.tile_pool(name="sbw", bufs=1) as sbw,      # weights/consts
            tc.tile_pool(name="sbpre", bufs=1) as sbpre,  # conv1x1 out (ext img)
            tc.tile_pool(name="sbin", bufs=3) as sbin,    # streamed conv inputs
            tc.tile_pool(name="sbqk", bufs=4) as sbqk,    # q/k chunk tiles
            tc.tile_pool(name="sbT", bufs=1) as sbT,      # qT/kT/v persistents
            tc.tile_pool(name="sbs", bufs=1) as sbs,      # small attn tiles
            tc.tile_pool(name="sbc", bufs=2) as sbc,      # trunk chunk pipeline
            tc.tile_pool(name="sbg", bufs=6) as sbg,      # gelu chunk tiles
            tc.tile_pool(name="pcv", bufs=4, space="PSUM") as pcv,
            tc.tile_pool(name="pdw", bufs=2, space="PSUM") as pdw,
            tc.tile_pool(name="pacc", bufs=1, space="PSUM") as pacc,
            tc.tile_pool(name="psm", bufs=1, space="PSUM") as psm,
        ):
            # ---------- weights ----------
            wq_t = sbw.tile([96, 2, CP], FP8, tag="wq", name="wq")
            wkv_t = sbw.tile([96, 2, 2 * CP], FP8, tag="wkv", name="wkv")
            nc.sync.dma_start(wq_t[:], d_wq.ap())
            qdw_t = [sbw.tile([128, 9, 128], FP8, tag=f"qdw{m}", name=f"qdw{m}") for m in range(2)]
            kvdw_t = [sbw.tile([128, 9, 128], FP8, tag=f"kvdw{m}", name=f"kvdw{m}") for m in range(4)]
            for m in range(2):
                nc.sync.dma_start(qdw_t[m][:], d_qdw[128 * m:128 * (m + 1)])

            def load_kv_weights():
                nc.sync.dma_start(wkv_t[:], d_wkv.ap())
                for m in range(4):
                    nc.sync.dma_start(kvdw_t[m][:], d_kvdw[128 * m:128 * (m + 1)])
            wlin_t = [sbw.tile([128, C], BF16, tag=f"wlin{m}", name=f"wlin{m}") for m in range(2)]
            wf1_t = [sbw.tile([s, 768], BF16, tag=f"wf1{i}", name=f"wf1{i}")
                     for i, (o, s) in enumerate(KB)]
            wf2_t = [sbw.tile([128, C], BF16, tag=f"wf2{k}", name=f"wf2{k}") for k in range(6)]
            wpr_t = [sbw.tile([s, C], BF16, tag=f"wpr{i}", name=f"wpr{i}")
                     for i, (o, s) in enumerate(KB)]
            tempb = sbw.tile([128, 2], F32, tag="tempb", name="tempb")
            alphav = sbw.tile([128, 1], F32, tag="alphav", name="alphav")
            gammav = sbw.tile([128, 1], F32, tag="gammav", name="gammav")
            id128 = sbw.tile([128, 128], F32, tag="id128", name="id128")
            idrep = sbw.tile([128, 64], F32, tag="idrep", name="idrep")

            def load_trunk_weights():
                for m in range(2):
                    nc.sync.dma_start(wlin_t[m][:], d_wlin[128 * m:128 * (m + 1), :])
                for i, (o, s) in enumerate(KB):
                    nc.sync.dma_start(wf1_t[i][:], d_wf1[o:o + s, :])
                for k in range(6):
                    nc.sync.dma_start(wf2_t[k][:], d_wf2[128 * k:128 * (k + 1), :])
                for i, (o, s) in enumerate(KB):
                    nc.sync.dma_start(wpr_t[i][:], d_wpr[o:o + s, :])
                nc.sync.dma_start(tempb[:], d_tempb.ap())
                nc.sync.dma_start(alphav[:], d_alpha.ap())
                nc.sync.dma_start(gammav[:], d_gamma.ap())
                nc.sync.dma_start(id128[:], d_id128.ap())
                nc.sync.dma_start(idrep[:], d_idrep.ap())

            # persistent attn-path results
            qT = [sbT.tile([128, 64, 112], BF16, tag=f"qT{p}", name=f"qT{p}") for p in range(2)]
            kT = [sbT.tile([128, 64, 112], BF16, tag=f"kT{p}", name=f"kT{p}") for p in range(2)]
            vband = [sbT.tile([128, NLOC], BF16, tag=f"v{m}", name=f"v{m}") for m in range(2)]
            sqp = [sbs.tile([128, NCK], F32, tag=f"sqp{i}", name=f"sqp{i}") for i in range(4)]
            for i in range(4):
                nc.vector.memset(sqp[i][:], 0.0)
            gacc = pacc.tile([112, 224], F32, tag="gacc", name="gacc")

            # ============ q/k/v production ============
            def conv_dw_path(src_dram, w_t, dw_tiles, n_mb, sink, m_off=0,
                             collect_mms=None):
                """192 -> n_mb*128 padded conv1x1 + depthwise 3x3 per band.
                sink(m, ck, psum_flat_ap) consumes each dwconv chunk."""
                for m in range(m_off, m_off + n_mb):
                    pre = sbpre.tile([128, ER, EC], FP8, tag="pre", name="pre")
                    pref = pre[:].rearrange("p a b -> p (a b)")
                    ci = 0
                    for g0, gn in CONV_GROUPS:
                        xc = sbin.tile([96, 2, GRP], FP8, tag="xin", name="xin")
                        nc.sync.dma_start(xc[:, :, :gn],
                                          src_dram[:, :, g0:g0 + gn])
                        for c0 in range(0, gn, 512):
                            cn = min(512, gn - c0)
                            ps = pcv.tile([128, 512], F32, tag="cv", name="cv")
                            mm = nc.tensor.matmul(
                                ps[:, :cn],
                                w_t[:, :, 128 * m:128 * (m + 1)],
                                xc[:, :, c0:c0 + cn],
                                start=True, stop=True, perf_mode=DRM)
                            if collect_mms is not None:
                                collect_mms.append(mm)
                            if ci % 2 == 0:
                                nc.vector.tensor_copy(pref[:, g0 + c0:g0 + c0 + cn],
                                                      ps[:, :cn])
                            else:
                                nc.scalar.copy(pref[:, g0 + c0:g0 + c0 + cn],
                                               ps[:, :cn])
                            ci += 1
                    for ck in range(NCK if KSUB >= 2 else 0):
                        r0 = 1 + 4 * ck
                        dp = pdw.tile([128, 4, 128], F32, tag="dw", name="dw")
                        for i, (ta, tb, stride) in enumerate(DW_PAIRS):
                            dra, dca = ta // 3 - 1, ta % 3 - 1
                            base = pre[:, r0 + dra:r0 + 4 + dra,
                                       1 + dca:129 + dca]
                            nc.tensor.matmul(
                                dp[:], dw_tiles[m][:, 2 * i:2 * i + 2, :],
                                _pair_ap(base, stride),
                                start=(i == 0), stop=False, perf_mode=DRM)
                        nc.tensor.matmul(
                            dp[:], dw_tiles[m][:, 8, :],
                            pre[:, r0:r0 + 4, 1:129],
                            start=False, stop=True)
                        sink(m, ck, dp[:].rearrange("p a b -> p (a b)"))

            def qk_sink(dstT, sq_idx):
                qcbig = [None]

                def sink(m, ck, flat):
                    j = ck % 4
                    if j == 0:
                        qcbig[0] = sbqk.tile([128, 2048], BF16, tag="qkc", name="qkc")
                    qc = qcbig[0][:, 512 * j:512 * (j + 1)]
                    if ck % 2 == 0:
                        nc.vector.tensor_copy(qc, flat)
                    else:
                        nc.scalar.copy(qc, flat)
                    if KSUB >= 3:
                        dmp = sbqk.tile([128, 512], F32, tag="dump", name="dump")
                        nc.scalar.activation(dmp[:], qc, AF.Square,
                                             accum_out=sqp[sq_idx + m][:, ck:ck + 1])
                    if KSUB >= 4 and j == 3:
                        nc.sync.dma_start_transpose(
                            dstT[m][:, 4 * ck - 12:4 * ck + 4, :],
                            qcbig[0][0:112, :])
                return sink

            load_kv_weights()
            conv_dw_path(d_xe, wq_t, qdw_t, 2, qk_sink(qT, 0))

            if STAGE >= 2:

                def v_sink(m, ck, flat):
                    dst = vband[m - 2]
                    if ck % 2 == 0:
                        nc.vector.tensor_copy(dst[:, ck * 512:(ck + 1) * 512], flat)
                    else:
                        nc.scalar.copy(dst[:, ck * 512:(ck + 1) * 512], flat)

                conv_dw_path(d_ye, wkv_t, kvdw_t, 2, qk_sink(kT, 2))
                load_trunk_weights()

            # ============ Gram accumulation ============
            gram_last = []
            for p in range(STAGE >= 2 and 2 or 0):
                for ck in range(64):
                    gmm = nc.tensor.matmul(gacc[:, 112 * p:112 * (p + 1)],
                                           qT[p][:, ck, :], kT[p][:, ck, :],
                                           start=(ck == 0), stop=(ck == 63))
                    if ck == 63:
                        gram_last.append(gmm)

            if STAGE < 3:
                oc0 = sbs.tile([128, NCK], F32, tag="oc0d", name="oc0d")
                nc.vector.tensor_copy(oc0[:], sqp[0][:])
                nc.sync.dma_start(d_out[0:128, 0:NCK], oc0[:])
            if STAGE >= 3:
                sqv = sbs.tile([128, 2], F32, tag="sqv", name="sqv")
                skv = sbs.tile([128, 2], F32, tag="skv", name="skv")
                for m in range(2):
                    nc.vector.tensor_reduce(sqv[:, m:m + 1], sqp[m][:],
                                            axis=mybir.AxisListType.X, op=OP.add)
                    nc.vector.tensor_reduce(skv[:, m:m + 1], sqp[2 + m][:],
                                            axis=mybir.AxisListType.X, op=OP.add)

                # ============ pair AllReduce ============
                gsb = sbs.tile([112, 224], F32, tag="gsb", name="gsb")
                nc.vector.tensor_copy(gsb[:], gacc[:])
                nc.scalar.dma_start(cc_in.ap()[:, 0:224], gsb[:])
                nc.scalar.dma_start(cc_in.ap()[:, 224:226], sqv[0:112, :])
                nc.scalar.dma_start(cc_in.ap()[:, 226:228], skv[0:112, :])
                nc.gpsimd.collective_compute(
                    "AllReduce", OP.add,
                    replica_groups=[[0, 1], [2, 3], [4, 5], [6, 7]],
                    ins=[cc_in.ap()], outs=[cc_out.ap()])
                # v(m2) here so it overlaps the AllReduce + softmax
                v_mms = []
                conv_dw_path(d_ye, wkv_t, kvdw_t, 1, v_sink, m_off=2,
                             collect_mms=v_mms)
                gg = sbs.tile([112, 224], F32, tag="gg", name="gg")
                sqg = sbs.tile([128, 2], F32, tag="sqg", name="sqg")
                skg = sbs.tile([128, 2], F32, tag="skg", name="skg")
                nc.vector.memset(sqg[:], 1.0)
                nc.vector.memset(skg[:], 1.0)
                nc.sync.dma_start(gg[:], cc_out.ap()[:, 0:224])
                nc.sync.dma_start(sqg[0:112, :], cc_out.ap()[:, 224:226])
                nc.sync.dma_start(skg[0:112, :], cc_out.ap()[:, 226:228])

                # ============ attention finalize ============
                def rsqrt_newton(tag, s_t):
                    sc = sbs.tile([128, 2], F32, tag=tag + "_c")
                    nc.vector.tensor_scalar_max(sc[:], s_t[:], 1e-24)
                    rt = sbs.tile([128, 2], F32, tag=tag + "_s")
                    nc.scalar.activation(rt[:], sc[:], AF.Sqrt)
                    r0 = sbs.tile([128, 2], F32, tag=tag + "_r0")
                    nc.vector.reciprocal(r0[:], rt[:])
                    rr = sbs.tile([128, 2], F32, tag=tag + "_rr")
                    nc.vector.tensor_tensor(out=rr[:], in0=r0[:], in1=r0[:], op=OP.mult)
                    t1_ = sbs.tile([128, 2], F32, tag=tag + "_t1")
                    nc.vector.scalar_tensor_tensor(out=t1_[:], in0=sc[:], scalar=-0.5,
                                                   in1=rr[:], op0=OP.mult, op1=OP.mult)
                    nc.vector.tensor_scalar_add(t1_[:], t1_[:], 1.5)
                    rv = sbs.tile([128, 2], F32, tag=tag)
                    nc.vector.tensor_tensor(out=rv[:], in0=r0[:], in1=t1_[:], op=OP.mult)
                    return rv

                rq = rsqrt_newton("rq", sqg)
                rk = rsqrt_newton("rk", skg)
                srow = sbs.tile([128, 2], F32, tag="srow", name="srow")
                nc.vector.tensor_tensor(out=srow[:], in0=rq[:], in1=tempb[:], op=OP.mult)

                srow_r, scol_r = [], []
                for p in range(2):
                    for src, lst, nm in ((srow, srow_r, "sr"), (rk, scol_r, "sc")):
                        fp = psm.tile([1, 112], F32, tag="sm", name="sm")
                        nc.tensor.transpose(fp[:], src[0:112, p:p + 1],
                                            id128[0:112, 0:112])
                        fr = sbs.tile([1, 112], F32R, tag=f"{nm}{p}", name=f"{nm}{p}")
                        nc.vector.tensor_copy(fr[:], fp[:])
                        lst.append(fr)

                attnT = [sbs.tile([112, 64], BF16, tag=f"attnT{p}", name=f"attnT{p}") for p in range(2)]
                for p in range(2):
                    nc.gpsimd.memset(attnT[p][:], 0.0)
                for p in range(2):
                    spair = psm.tile([112, 112], F32, tag="sm", name="sm")
                    nc.tensor.matmul(spair[:], srow_r[p][:], scol_r[p][:],
                                     start=True, stop=True)
                    lg = sbs.tile([112, 112], F32, tag="lg", name="lg")
                    nc.vector.tensor_tensor(out=lg[:], in0=gg[:, 112 * p:112 * (p + 1)],
                                            in1=spair[:], op=OP.mult)
                    at16 = sbs.tile([112, 112], BF16, tag="at16", name="at16")
                    for e in range(2):
                        sl = slice(64 * e, 64 * e + 48)
                        mx = sbs.tile([112, 1], F32, tag="mx", name="mx")
                        nc.vector.tensor_reduce(mx[sl, :], lg[sl, sl],
                                                axis=mybir.AxisListType.X, op=OP.max)
                        exh = sbs.tile([112, 112], F32, tag="exh", name="exh")
                        nc.vector.tensor_scalar(out=exh[sl, 0:48], in0=lg[sl, sl],
                                                scalar1=mx[sl, :], scalar2=None,
                                                op0=OP.subtract)
                        ex2 = sbs.tile([112, 112], F32, tag="ex2", name="ex2")
                        den = sbs.tile([112, 1], F32, tag="den", name="den")
                        nc.scalar.activation(ex2[sl, 0:48], exh[sl, 0:48], AF.Exp,
                                             accum_out=den[sl, :])
                        rc0 = sbs.tile([112, 1], F32, tag="rc0", name="rc0")
                        nc.vector.reciprocal(rc0[sl, :], den[sl, :])
                        nt = sbs.tile([112, 1], F32, tag="nt", name="nt")
                        nc.vector.tensor_tensor(out=nt[sl, :], in0=den[sl, :],
                                                in1=rc0[sl, :], op=OP.mult)
                        nc.vector.tensor_scalar(out=nt[sl, :], in0=nt[sl, :],
                                                scalar1=-1.0, scalar2=2.0,
                                                op0=OP.mult, op1=OP.add)
                        rc1 = sbs.tile([112, 1], F32, tag="rc1", name="rc1")
                        nc.vector.tensor_tensor(out=rc1[sl, :], in0=rc0[sl, :],
                                                in1=nt[sl, :], op=OP.mult)
                        nc.vector.tensor_scalar(out=at16[sl, 0:48], in0=ex2[sl, 0:48],
                                                scalar1=rc1[sl, :], scalar2=None,
                                                op0=OP.mult)
                        nc.sync.dma_start(d_attn[p, e], at16[sl, 0:48])
                        nc.sync.dma_start(
                            attnT[p][sl, 0:48],
                            d_attn[p, e].rearrange("a b -> b a"))

                conv_dw_path(d_ye, wkv_t, kvdw_t, 1, v_sink, m_off=3,
                             collect_mms=v_mms)
                for vm in v_mms:
                    for gl in gram_last:
                        _add_dep_helper(vm.ins, gl.ins, sync=False,
                                        reason="v after gram covers allreduce")

                if STAGE < 4:
                    ocx = sbs.tile([112, 64], F32, tag="ocx", name="ocx")
                    nc.vector.tensor_copy(ocx[:], attnT[0][:])
                    nc.sync.dma_start(d_out[0:112, 0:64], ocx[:])
                # ============ per-chunk trunk ============
                for ck in range(NCK if STAGE >= 4 else 0):
                    c0 = ck * 512
                    zp = [pdw.tile([128, 512], F32, tag="dw", name="dw") for _ in range(2)]
                    for p in range(2):
                        for e in range(2):
                            sl = slice(64 * e, 64 * e + 48)
                            osl = slice(64 * e, 64 * e + 64)
                            nc.tensor.matmul(zp[p][osl, :], attnT[p][sl, :],
                                             vband[p][sl, c0:c0 + 512],
                                             start=True, stop=True)
                    zc = [sbc.tile([128, 512], BF16, tag=f"zc{m}", name=f"zc{m}") for m in range(2)]
                    nc.vector.tensor_copy(zc[0][:], zp[0][:])
                    nc.scalar.copy(zc[1][:], zp[1][:])
                    tp = [pcv.tile([128, 512], F32, tag="cv", name="cv") for _ in range(2)]
                    for mi, (mo, ms) in enumerate(KB):
                        for k2 in range(2):
                            nc.tensor.matmul(tp[mi][:ms, :],
                                             wlin_t[k2][:, mo:mo + ms], zc[k2][:],
                                             start=(k2 == 0), stop=(k2 == 1))
                    ycn = sbc.tile([128, 2, 512], F32, tag="ycn", name="ycn")
                    for mi, (mo, ms) in enumerate(KB):
                        nc.sync.dma_start(ycn[:ms, mi, :], d_yc[mo:mo + ms, c0:c0 + 512])
                    t1c = [sbc.tile([s, 512], BF16, tag=f"t1c{i}", name=f"t1c{i}")
                           for i, (o, s) in enumerate(KB)]
                    for mi, (mo, ms) in enumerate(KB):
                        nc.vector.scalar_tensor_tensor(
                            out=t1c[mi][:], in0=ycn[:ms, mi, :], scalar=alphav[:ms, :],
                            in1=tp[mi][:ms, :], op0=OP.mult, op1=OP.add)
                    gc = [sbg.tile([128, 512], BF16, tag="gc", name="gc") for _ in range(6)]
                    for mt in range(6):
                        fp1 = pcv.tile([128, 512], F32, tag="cv", name="cv")
                        for i in range(2):
                            nc.tensor.matmul(fp1[:], wf1_t[i][:, 128 * mt:128 * (mt + 1)],
                                             t1c[i][:], start=(i == 0), stop=(i == 1))
                        nc.scalar.activation(gc[mt][:], fp1[:], AF.Gelu)
                    t2c = [sbc.tile([s, 512], BF16, tag=f"t2c{i}", name=f"t2c{i}")
                           for i, (o, s) in enumerate(KB)]
                    for mi, (mo, ms) in enumerate(KB):
                        fp2 = pcv.tile([128, 512], F32, tag="cv", name="cv")
                        for k in range(6):
                            nc.tensor.matmul(fp2[:ms, :], wf2_t[k][:, mo:mo + ms],
                                             gc[k][:], start=(k == 0), stop=(k == 5))
                        nc.vector.scalar_tensor_tensor(
                            out=t2c[mi][:], in0=t1c[mi][:], scalar=gammav[:ms, :],
                            in1=fp2[:ms, :], op0=OP.mult, op1=OP.add)
                    for mi, (mo, ms) in enumerate(KB):
                        pp = pcv.tile([128, 512], F32, tag="cv", name="cv")
                        for i in range(2):
                            nc.tensor.matmul(pp[:ms, :], wpr_t[i][:, mo:mo + ms],
                                             t2c[i][:], start=(i == 0), stop=(i == 1))
                        if DIRECT_PSUM_OUT:
                            nc.sync.dma_start(d_out[mo:mo + ms, c0:c0 + 512],
                                              pp[:ms, :])
                        else:
                            oc = sbc.tile([128, 512], F32, tag=f"oc{mi}", name=f"oc{mi}")
                            nc.scalar.copy(oc[:ms, :], pp[:ms, :])
                            nc.sync.dma_start(d_out[mo:mo + ms, c0:c0 + 512],
                                              oc[:ms, :])

    nc.compile()
    return nc


_NC = None


def _get_nc():
    global _NC
    if _NC is None:
        _NC = build_nc()
    return _NC


def _prep_weights(q_w, q_dw_w, kv_w, kv_dw_w, linear_w, proj_w, ffn1_w, ffn2_w,
                  temperature, alpha, beta, gamma, delta):
    def pad_oc(w):  # [192 real oc, ic] -> [ic, 256 padded oc]
        out = np.zeros((C, CP), np.float32)
        for h in range(HEADS):
            out[:, CPH * h:CPH * h + CH] = w[CH * h:CH * (h + 1), :].T
        return out

    wq = pad_oc(np.asarray(q_w, np.float32)) * 8.0
    kv = np.asarray(kv_w, np.float32)
    wkv = np.concatenate([pad_oc(kv[:C]), pad_oc(kv[C:])], axis=1) * 8.0

    # [192,1,3,3] -> [256, 9, 128] diag, slots = DW_PAIRS order + center
    slot_tap = [0, 2, 3, 5, 6, 8, 1, 7, 4]

    def pad_dw(w):
        out = np.zeros((CP, 9, 128), np.float32)
        for h in range(HEADS):
            for j in range(CH):
                cp = CPH * h + j
                taps = w[CH * h + j, 0].reshape(9)
                for s, t in enumerate(slot_tap):
                    out[cp, s, cp % 128] = taps[t]
        return out * 32.0

    qdw = pad_dw(np.asarray(q_dw_w, np.float32))
    kvd = np.asarray(kv_dw_w, np.float32)
    kvdw = np.concatenate([pad_dw(kvd[:C]), pad_dw(kvd[C:])], axis=0)

    lin = np.asarray(linear_w, np.float32) * (float(beta) / 256.0)
    wlin = np.zeros((CP, C), np.float32)
    for h in range(HEADS):
        wlin[CPH * h:CPH * h + CH, :] = lin[:, CH * h:CH * (h + 1)].T

    wf1 = np.asarray(ffn1_w, np.float32).T.copy()
    wf2 = (np.asarray(ffn2_w, np.float32) * float(delta)).T.copy()
    wpr = np.asarray(proj_w, np.float32).T.copy()

    tempb = np.zeros((128, 2), np.float32)
    tv = np.asarray(temperature, np.float32).reshape(HEADS)
    for h in range(HEADS):
        tempb[64 * (h % 2):64 * (h % 2) + 64, h // 2] = tv[h]

    alphav = np.full((128, 1), float(alpha), np.float32)
    gammav = np.full((128, 1), float(gamma), np.float32)
    id128 = np.eye(128, dtype=np.float32)
    idrep = np.zeros((128, 64), np.float32)
    for p_ in range(128):
        idrep[p_, p_ % 64] = 1.0

    return {
        "wq": wq.reshape(2, 96, CP).transpose(1, 0, 2).astype(f8),
        "wkv": wkv.reshape(2, 96, 2 * CP).transpose(1, 0, 2).astype(f8),
        "qdw": qdw.astype(f8), "kvdw": kvdw.astype(f8),
        "wlin": wlin.astype(bf16), "wf1": wf1.astype(bf16), "wf2": wf2.astype(bf16), "wpr": wpr.astype(bf16),
        "tempb": tempb, "alpha": alphav, "gamma": gammav,
        "id128": id128, "idrep": idrep,
    }


def _make_in_maps(x, y, shared):
    in_maps = []
    for c in range(N_CORES):
        bi, s = c // 2, c % 2
        r0 = s * HLOC
        xe = np.zeros((C, ER, EC), np.float32)
        ye = np.zeros((C, ER, EC), np.float32)
        rlo, rhi = max(r0 - 1, 0), min(r0 + HLOC + 1, H)
        elo = rlo - (r0 - 1)
        xe[:, elo:elo + (rhi - rlo), 1:129] = x[bi, :, rlo:rhi, :]
        ye[:, elo:elo + (rhi - rlo), 1:129] = y[bi, :, rlo:rhi, :]
        m = dict(shared)
        m["xe"] = xe.reshape(2, 96, NEXT).transpose(1, 0, 2).astype(f8)
        m["ye"] = ye.reshape(2, 96, NEXT).transpose(1, 0, 2).astype(f8)
        m["yc"] = y[bi, :, r0:r0 + HLOC, :].reshape(C, NLOC).astype(np.float32)
        in_maps.append(m)
    return in_maps


def kernel(**inputs):
    x = np.asarray(inputs["x"], np.float32)
    y = np.asarray(inputs["y"], np.float32)
    shared = _prep_weights(
        inputs["q_w"], inputs["q_dw_w"], inputs["kv_w"], inputs["kv_dw_w"],
        inputs["linear_w"], inputs["proj_w"], inputs["ffn1_w"], inputs["ffn2_w"],
        inputs["temperature"], inputs["alpha"], inputs["beta"],
        inputs["gamma"], inputs["delta"])

    in_maps = _make_in_maps(x, y, shared)

    nc = _get_nc()
    res = run_bass_kernel_spmd(nc, in_maps, list(range(N_CORES)))
    out = np.empty((B, C, H, W), np.float32)
    for c in range(N_CORES):
        bi, s = c // 2, c % 2
        out[bi, :, s * HLOC:(s + 1) * HLOC, :] = \
            res.results[c]["out"].reshape(C, HLOC, W)
    return out

